# revision 2
# baseline (speedup 1.0000x reference)
"""Trainium2 Bass kernel for DownwardPropagationDirect.

out[b, l] = sum_c input_flux[b, c] * prod_{j<l} t_direct[b, j, c]   (l = 0..L)

Pure data parallel over the batch: each of the 8 cores handles B/8 rows.
Per core, rows are tiled 128-to-a-partition, 16 row-chunks (2048 rows)
per group. t_direct streams in 4.7 MB pieces (15 layers x 16 chunks) on
HWDGE DMA. The running flux vector f (48 channels per row) ping-pongs
through a small tile pool; the Vector engine does the cumprod multiplies
and the final per-layer sums, while GPSIMD takes the first stage of each
channel reduction (48 -> 24 pairwise add) to keep the Vector engine under
the DMA roofline.
"""

import numpy as np

import concourse.bacc as bacc
import concourse.tile as tile
from concourse import mybir
from concourse.bass_utils import run_bass_kernel_spmd

B, L, C = 131072, 60, 48
N_CORES = 8
BS = B // N_CORES           # rows per core = 16384
P = 128                     # SBUF partitions
T = 16                      # 128-row chunks per group
ROWS_G = P * T              # rows per group = 2048
G = BS // ROWS_G            # groups per core = 8
LP = 15                     # layers per t-piece
NPIECE = L // LP            # pieces per group = 4

F32 = mybir.dt.float32


def build_kernel(reps: int = 1):
    nc = bacc.Bacc("TRN2", target_bir_lowering=False, debug=False,
                   num_devices=N_CORES)
    t_in = nc.dram_tensor("t_direct", [BS, L, C], F32, kind="ExternalInput").ap()
    flux_in = nc.dram_tensor("input_flux", [BS, C], F32, kind="ExternalInput").ap()
    out_t = nc.dram_tensor("out", [BS, L + 1], F32, kind="ExternalOutput").ap()

    # row = g*ROWS_G + r*128 + p
    t_re = t_in.rearrange("(g r p) l c -> g p r (l c)", g=G, r=T, p=P)
    flux_re = flux_in.rearrange("(g r p) c -> g p r c", g=G, r=T, p=P)
    out_re = out_t.rearrange("(g r p) l -> g p r l", g=G, r=T, p=P)

    X = mybir.AxisListType.X
    H = C // 2

    with tile.TileContext(nc) as tc:
        with tc.tile_pool(name="tpool", bufs=2) as tpool, \
             tc.tile_pool(name="fpool", bufs=8) as fpool, \
             tc.tile_pool(name="hpool", bufs=8) as hpool, \
             tc.tile_pool(name="fluxpool", bufs=2) as fluxpool, \
             tc.tile_pool(name="outpool", bufs=2) as outpool:
            for rep in range(reps):
                for g in range(G):
                    flux_tile = fluxpool.tile([P, T, C], F32,
                                              name=f"flux_{rep}_{g}", tag="flux")
                    nc.sync.dma_start(out=flux_tile[:], in_=flux_re[g])
                    o_tile = outpool.tile([P, T, L + 1], F32,
                                          name=f"o_{rep}_{g}", tag="o")
                    h0 = hpool.tile([P, T, H], F32, name=f"h_{rep}_{g}_0", tag="h")
                    nc.gpsimd.tensor_add(out=h0[:], in0=flux_tile[:, :, 0:H],
                                         in1=flux_tile[:, :, H:C])
                    nc.vector.reduce_sum(out=o_tile[:, :, 0:1], in_=h0[:], axis=X)
                    prev = flux_tile
                    for k in range(NPIECE):
                        tpiece = tpool.tile([P, T, LP * C], F32,
                                            name=f"t_{rep}_{g}_{k}", tag="t")
                        nc.sync.dma_start(
                            out=tpiece[:],
                            in_=t_re[g][:, :, k * LP * C:(k + 1) * LP * C])
                        for j in range(LP):
                            l = k * LP + j
                            f = fpool.tile([P, T, C], F32,
                                           name=f"f_{rep}_{g}_{l}", tag="f")
                            nc.vector.tensor_mul(
                                out=f[:], in0=prev[:],
                                in1=tpiece[:, :, j * C:(j + 1) * C])
                            h = hpool.tile([P, T, H], F32,
                                           name=f"h_{rep}_{g}_{l + 1}", tag="h")
                            nc.gpsimd.tensor_add(out=h[:], in0=f[:, :, 0:H],
                                                 in1=f[:, :, H:C])
                            nc.vector.reduce_sum(
                                out=o_tile[:, :, l + 1:l + 2], in_=h[:], axis=X)
                            prev = f
                    nc.sync.dma_start(out=out_re[g], in_=o_tile[:])

    nc.compile()
    return nc


_NC_CACHE = {}


def kernel(input_flux: np.ndarray, t_direct: np.ndarray,
           _reps: int = 1) -> np.ndarray:
    if _reps not in _NC_CACHE:
        _NC_CACHE[_reps] = build_kernel(_reps)
    nc = _NC_CACHE[_reps]

    input_flux = np.ascontiguousarray(input_flux, dtype=np.float32)
    t_direct = np.ascontiguousarray(t_direct, dtype=np.float32)

    in_maps = []
    for i in range(N_CORES):
        sl = slice(i * BS, (i + 1) * BS)
        in_maps.append({
            "t_direct": t_direct[sl],
            "input_flux": input_flux[sl],
        })

    res = run_bass_kernel_spmd(nc, in_maps, list(range(N_CORES)))
    return np.concatenate([res.results[i]["out"] for i in range(N_CORES)], axis=0)
